# revision 35
# baseline (speedup 1.0000x reference)
"""Trainium2 Bass kernel for DecoderMultiHeadAttention (B=2, T=2048, C=768, H=12).

Sharding: 8 cores = 2 batches x 4 head-groups (3 heads each).
Per core: K,V projections for its head group (f32r), RoPE in f16 on
deinterleaved (real|imag) head layout, causal flash-style attention with
transposed score layout in f16, per-T-split AllGather within each batch
group of 4 cores overlapped with later attention splits, then a
column-sharded output projection consuming the gathered f16 tiles
directly as matmul lhsT (output in [T, 192] f16).

Host-side weight prep: W_att K/V columns and W_proj rows are permuted so
each head's dim order is [re(32)|im(32)] — keeps every RoPE vector op on
unit-stride 32-col blocks (DVE 4x f16 mode). Scores are invariant to the
shared K/Q permutation; W_proj rows are permuted to match V's order.

Note: the reference uses q = rope(v) (faithful source bug), so the
q-chunk of W_att (columns C..2C) is never used and is not computed.
"""

import sys

_REPO = "/opt/trn_rl_repo"
if _REPO not in sys.path:
    sys.path.insert(0, _REPO)

import numpy as np

import concourse.bass as bass
import concourse.mybir as mybir
import concourse.tile as tile
from concourse import bacc
from concourse.bass_utils import run_bass_kernel_spmd
from concourse.masks import make_identity

B, T, C, H = 2, 2048, 768, 12
D = C // H            # 64
N_CORES = 8
G = 4                 # head groups
HG = H // G           # 3 heads per group
CG = HG * D           # 192 output columns per group
NT = T // 128         # 16 t-chunks
NCC = C // 128        # 6 c-chunks
TQ = 512              # q block width
F32 = mybir.dt.float32
F32R = mybir.dt.float32r
F16 = mybir.dt.float16
EXP = mybir.ActivationFunctionType.Exp
SCALE = float(D) ** -0.5
SPLITS = [(0, 512), (512, 512), (1024, 512), (1536, 256),
          (1792, 128), (1920, 128)]


def _body(nc, tc, x, wkv, wp, bp, cos3, sin3, out_t, sim_variant=False, reps=1):
    with tc.tile_pool(name="const", bufs=1) as cp:
        ident = cp.tile([128, 128], F32)
        make_identity(nc, ident[:])
        identR = cp.tile([128, 128], F32R)
        nc.scalar.copy(identR[:], ident[:])
        identH = cp.tile([128, 128], F16)
        nc.vector.tensor_copy(identH[:], ident[:])
        # tri[p, f] = 1.0 if f >= p else 0.0  (keep tq >= tk in diagonal blocks)
        tri = cp.tile([128, 128], F32)
        nc.gpsimd.memset(tri[:], 1.0)
        nc.gpsimd.affine_select(
            out=tri[:], in_=tri[:], compare_op=mybir.AluOpType.is_ge,
            fill=0.0, base=0, pattern=[[1, 128]], channel_multiplier=-1)
        triH = cp.tile([128, 128], F16)
        nc.vector.tensor_copy(triH[:], tri[:])

        # cdup/sdup: [128, NT*384] f16, deinterleaved + duplicated across the
        # K and Q halves: chunk i, half f, head h, cols i*384 + f*192 + h*64
        # + {j, 32+j} = table[i*128+p, j] — one 384-wide mul ropes K and Q
        cos_sb = cp.tile([128, NT * 384], F16)
        sin_sb = cp.tile([128, NT * 384], F16)
        # K,V weights in f16 (host-converted); stage-1 matmuls run f16
        wkv_sb = cp.tile([128, NCC * 2 * CG], F16)
        nc.scalar.dma_start(
            wkv_sb[:].rearrange("p (n m) -> p n m", n=NCC),
            wkv.rearrange("(n p) m -> p n m", p=128))
        # projection weights / bias in f16 (host-converted)
        wp_sb = cp.tile([128, NCC * CG], F16)
        bp_sb = cp.tile([1, CG], F16)
        ones1 = cp.tile([1, 128], F16)
        nc.gpsimd.memset(ones1[:], 1.0)

        # persistent per-head [D, T] f16 tensors: heads 0,1 packed; head 2
        # of K and Q share one tile (K rows 0:64, Q rows 64:128)
        kT01 = cp.tile([128, T], F16)
        qT01 = cp.tile([128, T], F16)
        kT2 = cp.tile([64, T], F16)
        qT2 = cp.tile([64, T], F16)
        # V in [T, D] layout with a ones column per head: per t-chunk i,
        # cols [i*195 + h*65 : +64] = V_h (deinterleaved dim order),
        # col +64 = 1.0
        vaug = cp.tile([128, NT * (HG * 65)], F16)
        ones48 = cp.tile([128, NT * HG], F16)
        nc.gpsimd.memset(ones48[:], 1.0)
        nc.vector.tensor_copy(
            vaug[:].rearrange("p (k c) -> p k c", c=65)[:, :, 64], ones48[:])
        # attention output, transposed [CG, bw] f16, one tile pair per split
        NSP = len(SPLITS)
        oT01h = [cp.tile([128, bw], F16, name=f"oT01h{k}")
                 for k, (bs_, bw) in enumerate(SPLITS)]
        oT2h = [cp.tile([64, bw], F16, name=f"oT2h{k}")
                for k, (bs_, bw) in enumerate(SPLITS)]

        dp = tc.alloc_tile_pool(name="dram", bufs=1, space="DRAM")
        ag_in = [dp.tile([CG, bw], F16, name=f"agin{k}")
                 for k, (bs_, bw) in enumerate(SPLITS)]
        ag_out = [dp.tile([G * CG, bw], F16, name=f"agout{k}")
                  for k, (bs_, bw) in enumerate(SPLITS)]

        for _rep in range(reps):
            # ---- Stage 1: KV projection + RoPE + transposes ----
            with tc.tile_pool(name="s1", bufs=3) as s1, \
                 tc.tile_pool(name="s2", bufs=2) as s2, \
                 tc.tile_pool(name="s3", bufs=1) as s3:
              with tc.tile_pool(name="s1ps", bufs=2, space="PSUM") as s1ps:
                for i in range(NT):
                    x_sb = s1.tile([128, C], F32R, tag="x", bufs=4)
                    nc.sync.dma_start(x_sb[:],
                                      x[i * 128:(i + 1) * 128, :].bitcast(F32R))
                    if i % 2 == 0 and i < 16:
                        # rope tables in eighths on the scalar HWDGE queue,
                        # spread so x-chunk loads are not starved early
                        qq = slice((i // 2) * (NT // 8) * 384,
                                   ((i // 2) + 1) * (NT // 8) * 384)
                        nc.scalar.dma_start(cos_sb[:, qq], cos3[:, qq])
                        nc.scalar.dma_start(sin_sb[:, qq], sin3[:, qq])
                    # cast to f16 on the (idle-in-stage-1) Pool engine, then
                    # batched PE transpose of the whole [128, C] chunk at the
                    # f16 transpose rate
                    xh_sb = s1.tile([128, C], F16, tag="xh", bufs=3)
                    nc.gpsimd.tensor_copy(xh_sb[:], x_sb[:].bitcast(F32))
                    xtp = s1ps.tile([128, C], F16, tag="xtp", bufs=2)
                    for c in range(NCC):
                        nc.tensor.transpose(xtp[:, c * 128:(c + 1) * 128],
                                            xh_sb[:, c * 128:(c + 1) * 128],
                                            identH[:])
                    xT_sb = s1.tile([128, C], F16, tag="xTs")
                    nc.vector.tensor_copy(xT_sb[:], xtp[:])
                    kv_ps = s1ps.tile([128, 2 * CG], F32, tag="kv")
                    for c in range(NCC):
                        nc.tensor.matmul(
                            kv_ps[:], xT_sb[:, c * 128:(c + 1) * 128],
                            wkv_sb[:, c * 2 * CG:(c + 1) * 2 * CG],
                            start=(c == 0), stop=(c == NCC - 1))
                    # K|V to SBUF in f16 (all downstream compute is f16)
                    kv_sb = s1.tile([128, 2 * CG], F16, tag="kvs")
                    nc.scalar.copy(kv_sb[:], kv_ps[:])

                    # RoPE on deinterleaved [re(32)|im(32)] blocks, K and Q
                    # halves in one 384-wide op each:
                    # K half -> kq[:, 0:CG], Q = rope(V) half -> kq[:, CG:2CG]
                    kq_sb = s1.tile([128, 2 * CG], F16, tag="kq")
                    cS = cos_sb[:, i * 2 * CG:(i + 1) * 2 * CG]
                    sS = sin_sb[:, i * 2 * CG:(i + 1) * 2 * CG]
                    a_sb = s1.tile([128, 2 * CG], F16, tag="ra")
                    b_sb = s1.tile([128, 2 * CG], F16, tag="rb")
                    nc.vector.tensor_mul(a_sb[:], kv_sb[:], cS)
                    nc.vector.tensor_mul(b_sb[:], kv_sb[:], sS)
                    a6 = a_sb[:].rearrange("p (h c) -> p h c", h=2 * HG)
                    b6 = b_sb[:].rearrange("p (h c) -> p h c", h=2 * HG)
                    o6 = kq_sb[:].rearrange("p (h c) -> p h c", h=2 * HG)
                    nc.vector.tensor_sub(o6[:, :, 0:32],
                                         a6[:, :, 0:32], b6[:, :, 32:64])
                    nc.vector.tensor_add(o6[:, :, 32:64],
                                         b6[:, :, 0:32], a6[:, :, 32:64])

                    # V (unroped, deinterleaved dims) into vaug [T, 65*3]
                    vdst = vaug[:, i * 195:(i + 1) * 195] \
                        .rearrange("p (h c) -> p h c", h=HG)[:, :, 0:64]
                    vsrc = kv_sb[:, CG:2 * CG].rearrange("p (h c) -> p h c", h=HG)
                    nc.gpsimd.tensor_copy(vdst, vsrc)

                    # transpose roped K and Q into [D, T] per-head layouts
                    tp = s1ps.tile([128, 4 * 128], F16, tag="tp")
                    nc.tensor.transpose(tp[:, 0:128], kq_sb[:, 0:128], identH[:])
                    nc.tensor.transpose(tp[:, 128:256], kq_sb[:, 192:320],
                                        identH[:])
                    nc.tensor.transpose(tp[0:64, 256:384], kq_sb[:, 128:192],
                                        identH[:])
                    nc.tensor.transpose(tp[0:64, 384:512], kq_sb[:, 320:384],
                                        identH[:])
                    ts_ = slice(i * 128, (i + 1) * 128)
                    nc.scalar.copy(kT01[:, ts_], tp[:, 0:128])
                    nc.scalar.copy(qT01[:, ts_], tp[:, 128:256])
                    nc.scalar.copy(kT2[:, ts_], tp[0:64, 256:384])
                    nc.scalar.copy(qT2[:, ts_], tp[0:64, 384:512])

              # projection weights load during stage 2
              nc.sync.dma_start(wp_sb[:].rearrange("p (n m) -> p n m", n=NCC),
                                wp.rearrange("(n p) m -> p n m", p=128))
              nc.sync.dma_start(bp_sb[:], bp)

              with tc.tile_pool(name="s23ps", bufs=2, space="PSUM") as s2ps:
                  # software pipeline over (split, head) pairs: pair j's score
                  # matmuls + exp issue BEFORE pair j-1's o-matmuls, so PE
                  # fills the exp latency with useful score work instead of
                  # stalling on the ACT engine per pack.
                  pairs = [(b, h) for b in range(len(SPLITS))
                           for h in range(HG)]

                  def _packs(bs, bw):
                      nblk = (bs + bw) // 128
                      packs, cur, w = [], [], 0
                      for t in range(nblk):
                          diag = t * 128 >= bs
                          col0 = t * 128 - bs if diag else 0
                          ncols = bw - col0
                          if w + ncols > 2 * TQ:
                              packs.append(cur)
                              cur, w = [], 0
                          cur.append((t, col0, ncols, w, diag))
                          w += ncols
                      if cur:
                          packs.append(cur)
                      return packs

                  def issue_scores(j):
                      b, h = pairs[j]
                      bs, bw = SPLITS[b]
                      kT = (kT01[0:64], kT01[64:128], kT2[0:64])[h]
                      qT = (qT01[0:64], qT01[64:128], qT2[0:64])[h]
                      out = []
                      for pk in _packs(bs, bw):
                          pw = sum(c[2] for c in pk)
                          s_ps = s2ps.tile([128, 2 * TQ], F32, tag="s", bufs=2)
                          wei = s2.tile([128, 2 * TQ], F16, tag="wei", bufs=8)
                          for t, col0, ncols, off, diag in pk:
                              nc.tensor.matmul(
                                  s_ps[:, off:off + ncols],
                                  kT[:, t * 128:(t + 1) * 128],
                                  qT[:, bs + col0:bs + bw],
                                  start=True, stop=True)
                          nc.scalar.activation(wei[:, 0:pw], s_ps[:, 0:pw],
                                               EXP, scale=SCALE)
                          out.append((pk, wei))
                      return out

                  def issue_o(j, scored):
                      b, h = pairs[j]
                      bs, bw = SPLITS[b]
                      oT = (oT01h[b][0:64], oT01h[b][64:128], oT2h[b][0:64])[h]
                      nblk = (bs + bw) // 128
                      o_ps = s2ps.tile([65, TQ], F32, tag="o", bufs=3)
                      for pk, wei in scored:
                          for t, col0, ncols, off, diag in pk:
                              if diag:
                                  nc.gpsimd.tensor_mul(wei[:, off:off + 128],
                                                       wei[:, off:off + 128],
                                                       triH[:])
                              va = t * 195 + h * 65
                              nc.tensor.matmul(
                                  o_ps[0:65, col0:bw], vaug[:, va:va + 65],
                                  wei[:, off:off + ncols],
                                  start=(t == 0), stop=(t == nblk - 1))
                      recip = s2.tile([1, TQ], F32, tag="recip", bufs=2)
                      nc.vector.reciprocal(recip[:, 0:bw], o_ps[64:65, 0:bw])
                      rb = s2.tile([64, TQ], F32, tag="rbd", bufs=2)
                      nc.gpsimd.partition_broadcast(rb[:, 0:bw],
                                                    recip[:, 0:bw])
                      nc.vector.tensor_mul(oT[:], o_ps[0:64, 0:bw],
                                           rb[:, 0:bw])
                      if h == HG - 1:
                          issue_stage3(b)

                  def issue_stage3(b):
                      # AllGather within the batch group, then column-sharded
                      # projection consuming the gathered f16 tiles as lhsT.
                      bs, bw = SPLITS[b]
                      nc.sync.dma_start(ag_in[b][0:64, :], oT01h[b][0:64])
                      nc.gpsimd.dma_start(ag_in[b][64:128, :], oT01h[b][64:128])
                      nc.scalar.dma_start(ag_in[b][128:CG, :], oT2h[b][:])
                      if sim_variant:
                          qs_ = (nc.sync, nc.scalar, nc.gpsimd, nc.sync)
                          for gg in range(G):
                              qs_[gg].dma_start(
                                  ag_out[b][gg * CG:(gg + 1) * CG, :],
                                  ag_in[b][:])
                      else:
                          nc.gpsimd.collective_compute(
                              "AllGather", mybir.AluOpType.bypass,
                              replica_groups=[[0, 1, 2, 3], [4, 5, 6, 7]],
                              ins=[ag_in[b][:].opt()], outs=[ag_out[b][:].opt()])
                      ntc = bw // 128
                      a_bf = s3.tile([128, NCC * TQ], F16, tag="abf", bufs=2)
                      nc.sync.dma_start(
                          a_bf[:, 0:NCC * bw].rearrange("p (n m) -> p n m", n=NCC),
                          ag_out[b][:].rearrange("(n p) m -> p n m", p=128))
                      o_sb = s3.tile([128, 4 * CG], F16, tag="osb", bufs=2)
                      for tc_ in range(ntc):
                          p_ps = s2ps.tile([128, CG], F32, tag="p", bufs=1)
                          for c in range(NCC):
                              nc.tensor.matmul(
                                  p_ps[:],
                                  a_bf[:, c * bw + tc_ * 128:
                                       c * bw + (tc_ + 1) * 128],
                                  wp_sb[:, c * CG:(c + 1) * CG],
                                  start=(c == 0), stop=False)
                          nc.tensor.matmul(p_ps[:], ones1[:], bp_sb[:],
                                           start=False, stop=True)
                          nc.vector.tensor_copy(
                              o_sb[:, tc_ * CG:(tc_ + 1) * CG], p_ps[:])
                      nc.sync.dma_start(
                          out_t[bs:bs + bw, :].rearrange("(n p) m -> p n m",
                                                         p=128),
                          o_sb[:, 0:ntc * CG].rearrange("p (n m) -> p n m",
                                                        n=ntc))

                  scored = {}
                  for j in range(len(pairs) + 1):
                      if j < len(pairs):
                          scored[j] = issue_scores(j)
                      if j >= 1:
                          issue_o(j - 1, scored.pop(j - 1))


def _build(sim_variant=False, reps=1):
    nc = bacc.Bacc("TRN2", target_bir_lowering=False, debug=False,
                   num_devices=1 if sim_variant else N_CORES,
                   enable_asserts=False)
    x = nc.dram_tensor("x", [T, C], F32, kind="ExternalInput").ap()
    wkv = nc.dram_tensor("wkv", [C, 2 * CG], F16, kind="ExternalInput").ap()
    wp = nc.dram_tensor("wp", [C, CG], F16, kind="ExternalInput").ap()
    bp = nc.dram_tensor("bp", [1, CG], F16, kind="ExternalInput").ap()
    cos3 = nc.dram_tensor("cos3", [128, NT * 384], F16, kind="ExternalInput").ap()
    sin3 = nc.dram_tensor("sin3", [128, NT * 384], F16, kind="ExternalInput").ap()
    out_t = nc.dram_tensor("out_t", [T, CG], F16, kind="ExternalOutput").ap()
    with tile.TileContext(nc) as tc:
        _body(nc, tc, x, wkv, wp, bp, cos3, sin3, out_t, sim_variant, reps)
    nc.compile()
    return nc


_NC = None


def _get_nc():
    global _NC
    if _NC is None:
        _NC = _build()
    return _NC


_EXEC = None


def _get_exec():
    global _EXEC
    if _EXEC is None:
        _EXEC = _make_exec(_get_nc())
    return _EXEC


def _make_exec(nc):
    """Reusable jitted SPMD executable (mirrors bass2jax.run_bass_via_pjrt's
    multi-core path)."""
    import jax
    from jax.experimental.shard_map import shard_map
    from jax.sharding import Mesh, PartitionSpec
    from concourse import bass2jax, mybir as _mybir

    bass2jax.install_neuronx_cc_hook()
    in_names, out_names, out_avals, zero_outs = [], [], [], []
    assert nc.dbg_addr is None
    pname = nc.partition_id_tensor.name if nc.partition_id_tensor else None
    for alloc in nc.m.functions[0].allocations:
        if not isinstance(alloc, _mybir.MemoryLocationSet):
            continue
        name = alloc.memorylocations[0].name
        if alloc.kind == "ExternalInput":
            if name != pname:
                in_names.append(name)
        elif alloc.kind == "ExternalOutput":
            out_names.append(name)
            shape = tuple(alloc.tensor_shape)
            dtype = _mybir.dt.np(alloc.dtype)
            out_avals.append(jax.core.ShapedArray(shape, dtype))
            zero_outs.append(np.zeros(shape, dtype))
    n_params = len(in_names)
    all_names = in_names + out_names
    if pname is not None:
        all_names = all_names + [pname]

    def _fn(*args):
        operands = list(args)
        if pname is not None:
            operands.append(bass2jax.partition_id_tensor())
        outs = bass2jax._bass_exec_p.bind(
            *operands,
            out_avals=tuple(out_avals),
            in_names=tuple(all_names),
            out_names=tuple(out_names),
            lowering_input_output_aliases=(),
            sim_require_finite=True,
            sim_require_nnan=True,
            nc=nc,
        )
        return tuple(outs)

    devices = jax.devices()[:N_CORES]
    mesh = Mesh(np.asarray(devices), ("core",))
    nin = n_params + len(out_names)
    donate = tuple(range(n_params, n_params + len(out_names)))
    sharded = jax.jit(
        shard_map(_fn, mesh=mesh,
                  in_specs=(PartitionSpec("core"),) * nin,
                  out_specs=(PartitionSpec("core"),) * len(out_names),
                  check_rep=False),
        donate_argnums=donate, keep_unused=True)

    def _zero_cat():
        return [np.zeros((N_CORES * z.shape[0], *z.shape[1:]), z.dtype)
                for z in zero_outs]

    return (sharded, in_names, out_names, out_avals, _zero_cat)


def _run_cached(in_maps):
    sharded, in_names, out_names, out_avals, zero_cat = _get_exec()
    concat_in = [np.concatenate([np.asarray(in_maps[c][n])
                                 for c in range(N_CORES)], axis=0)
                 for n in in_names]
    out_arrs = sharded(*concat_in, *zero_cat())
    return [
        {name: np.asarray(out_arrs[i]).reshape(N_CORES, *out_avals[i].shape)[c]
         for i, name in enumerate(out_names)}
        for c in range(N_CORES)
    ]


_PERM = np.concatenate([np.arange(0, 64, 2), np.arange(1, 64, 2)])  # [re|im]


def _prep_rope(r):
    # [T, 32] -> [128, NT*384] f16 deinterleaved and duplicated across the
    # K and Q halves: chunk i, half f, head h, cols i*384 + f*192 + h*64 +
    # {j, 32+j} both hold r[i*128+p, j]
    rr = r.reshape(NT, 128, 32).transpose(1, 0, 2)           # [128, NT, 32]
    rr = np.concatenate([rr, rr], axis=2)                    # [128, NT, 64]
    rr = np.broadcast_to(rr[:, :, None, :], (128, NT, 2 * HG, 64))
    return np.ascontiguousarray(rr.reshape(128, NT * 384), dtype=np.float16)


def _shard_inputs(x, rope_cos, rope_sin, W_att, W_proj, b_proj):
    x = np.ascontiguousarray(np.asarray(x, np.float32))
    W_att = np.asarray(W_att, np.float32)
    W_proj = np.asarray(W_proj, np.float32)
    b_proj = np.asarray(b_proj, np.float32)
    cos3 = _prep_rope(np.asarray(rope_cos, np.float32))
    sin3 = _prep_rope(np.asarray(rope_sin, np.float32))
    in_maps = []
    for r in range(N_CORES):
        b, g = divmod(r, G)
        c0 = g * CG
        # per-head deinterleave permutation of the group's 192 dims
        dperm = np.concatenate([h * 64 + _PERM for h in range(HG)])
        # full-C row permutation for W_proj: every group's dims are
        # deinterleaved in the gathered activation layout
        cperm = np.concatenate([g2 * CG + dperm for g2 in range(G)])
        wkv = np.ascontiguousarray(
            np.concatenate([W_att[:, c0:c0 + CG][:, dperm],
                            W_att[:, 2 * C + c0:2 * C + c0 + CG][:, dperm]],
                           axis=1)).astype(np.float16)
        in_maps.append({
            "x": x[b],
            "wkv": wkv,
            "wp": np.ascontiguousarray(
                W_proj[cperm, :][:, c0:c0 + CG]).astype(np.float16),
            "bp": np.ascontiguousarray(
                b_proj[c0:c0 + CG][None, :]).astype(np.float16),
            "cos3": cos3,
            "sin3": sin3,
        })
    return in_maps


def kernel(x, rope_cos, rope_sin, W_att, W_proj, b_proj, _run_kwargs=None):
    nc = _get_nc()
    in_maps = _shard_inputs(x, rope_cos, rope_sin, W_att, W_proj, b_proj)
    global _FIRST_CALL_DONE, _last_in_maps
    _last_in_maps = in_maps
    if not _FIRST_CALL_DONE:
        res = run_bass_kernel_spmd(nc, in_maps, core_ids=list(range(N_CORES)),
                                   **(_run_kwargs or {}))
        results = res.results
        kernel.last_results = res
        _FIRST_CALL_DONE = True
    else:
        results = _run_cached(in_maps)
    out = np.empty((B, T, C), np.float32)
    for r in range(N_CORES):
        b, g = divmod(r, G)
        out[b, :, g * CG:(g + 1) * CG] = results[r]["out_t"].astype(np.float32)
    return out


_FIRST_CALL_DONE = False


# revision 38
# speedup vs baseline: 1.0081x; 1.0081x over previous
"""Trainium2 Bass kernel for DecoderMultiHeadAttention (B=2, T=2048, C=768, H=12).

Sharding: 8 cores = 2 batches x 4 head-groups (3 heads each).
Per core: K,V projections for its head group (f32r), RoPE in f16 on
deinterleaved (real|imag) head layout, causal flash-style attention with
transposed score layout in f16, per-T-split AllGather within each batch
group of 4 cores overlapped with later attention splits, then a
column-sharded output projection consuming the gathered f16 tiles
directly as matmul lhsT (output in [T, 192] f16).

Host-side weight prep: W_att K/V columns and W_proj rows are permuted so
each head's dim order is [re(32)|im(32)] — keeps every RoPE vector op on
unit-stride 32-col blocks (DVE 4x f16 mode). Scores are invariant to the
shared K/Q permutation; W_proj rows are permuted to match V's order.

Note: the reference uses q = rope(v) (faithful source bug), so the
q-chunk of W_att (columns C..2C) is never used and is not computed.
"""

import sys

_REPO = "/opt/trn_rl_repo"
if _REPO not in sys.path:
    sys.path.insert(0, _REPO)

import numpy as np

import concourse.bass as bass
import concourse.mybir as mybir
import concourse.tile as tile
from concourse import bacc
from concourse.bass_utils import run_bass_kernel_spmd
from concourse.masks import make_identity

B, T, C, H = 2, 2048, 768, 12
D = C // H            # 64
N_CORES = 8
G = 4                 # head groups
HG = H // G           # 3 heads per group
CG = HG * D           # 192 output columns per group
NT = T // 128         # 16 t-chunks
NCC = C // 128        # 6 c-chunks
TQ = 512              # q block width
F32 = mybir.dt.float32
F32R = mybir.dt.float32r
F16 = mybir.dt.float16
EXP = mybir.ActivationFunctionType.Exp
SCALE = float(D) ** -0.5
SPLITS = [(0, 512), (512, 512), (1024, 512), (1536, 256), (1792, 256)]


def _body(nc, tc, x, wkv, wp, bp, cos3, sin3, out_t, sim_variant=False, reps=1):
    with tc.tile_pool(name="const", bufs=1) as cp:
        ident = cp.tile([128, 128], F32)
        make_identity(nc, ident[:])
        identR = cp.tile([128, 128], F32R)
        nc.scalar.copy(identR[:], ident[:])
        identH = cp.tile([128, 128], F16)
        nc.vector.tensor_copy(identH[:], ident[:])
        # tri[p, f] = 1.0 if f >= p else 0.0  (keep tq >= tk in diagonal blocks)
        tri = cp.tile([128, 128], F32)
        nc.gpsimd.memset(tri[:], 1.0)
        nc.gpsimd.affine_select(
            out=tri[:], in_=tri[:], compare_op=mybir.AluOpType.is_ge,
            fill=0.0, base=0, pattern=[[1, 128]], channel_multiplier=-1)
        triH = cp.tile([128, 128], F16)
        nc.vector.tensor_copy(triH[:], tri[:])

        # cdup/sdup: [128, NT*384] f16, deinterleaved + duplicated across the
        # K and Q halves: chunk i, half f, head h, cols i*384 + f*192 + h*64
        # + {j, 32+j} = table[i*128+p, j] — one 384-wide mul ropes K and Q
        cos_sb = cp.tile([128, NT * 384], F16)
        sin_sb = cp.tile([128, NT * 384], F16)
        # K,V weights in f16 (host-converted); stage-1 matmuls run f16
        wkv_sb = cp.tile([128, NCC * 2 * CG], F16)
        nc.scalar.dma_start(
            wkv_sb[:].rearrange("p (n m) -> p n m", n=NCC),
            wkv.rearrange("(n p) m -> p n m", p=128))
        # projection weights / bias in f16 (host-converted)
        wp_sb = cp.tile([128, NCC * CG], F16)
        bp_sb = cp.tile([1, CG], F16)
        ones1 = cp.tile([1, 128], F16)
        nc.gpsimd.memset(ones1[:], 1.0)

        # persistent per-head [D, T] f16 tensors: heads 0,1 packed; head 2
        # of K and Q share one tile (K rows 0:64, Q rows 64:128)
        kT01 = cp.tile([128, T], F16)
        qT01 = cp.tile([128, T], F16)
        kT2 = cp.tile([64, T], F16)
        qT2 = cp.tile([64, T], F16)
        # V in [T, D] layout with a ones column per head: per t-chunk i,
        # cols [i*195 + h*65 : +64] = V_h (deinterleaved dim order),
        # col +64 = 1.0
        vaug = cp.tile([128, NT * (HG * 65)], F16)
        ones48 = cp.tile([128, NT * HG], F16)
        nc.gpsimd.memset(ones48[:], 1.0)
        nc.vector.tensor_copy(
            vaug[:].rearrange("p (k c) -> p k c", c=65)[:, :, 64], ones48[:])
        # attention output, transposed [CG, bw] f16, one tile pair per split
        NSP = len(SPLITS)
        oT01h = [cp.tile([128, bw], F16, name=f"oT01h{k}")
                 for k, (bs_, bw) in enumerate(SPLITS)]
        oT2h = [cp.tile([64, bw], F16, name=f"oT2h{k}")
                for k, (bs_, bw) in enumerate(SPLITS)]

        dp = tc.alloc_tile_pool(name="dram", bufs=1, space="DRAM")
        ag_in = [dp.tile([CG, bw], F16, name=f"agin{k}")
                 for k, (bs_, bw) in enumerate(SPLITS)]
        ag_out = [dp.tile([G * CG, bw], F16, name=f"agout{k}")
                  for k, (bs_, bw) in enumerate(SPLITS)]

        for _rep in range(reps):
            # ---- Stage 1: KV projection + RoPE + transposes ----
            with tc.tile_pool(name="s1", bufs=3) as s1, \
                 tc.tile_pool(name="s2", bufs=2) as s2, \
                 tc.tile_pool(name="s3", bufs=1) as s3:
              with tc.tile_pool(name="s1ps", bufs=2, space="PSUM") as s1ps:
                for i in range(NT):
                    x_sb = s1.tile([128, C], F32R, tag="x", bufs=4)
                    nc.sync.dma_start(x_sb[:],
                                      x[i * 128:(i + 1) * 128, :].bitcast(F32R))
                    if i % 2 == 0 and i < 16:
                        # rope tables in eighths on the scalar HWDGE queue,
                        # spread so x-chunk loads are not starved early
                        qq = slice((i // 2) * (NT // 8) * 384,
                                   ((i // 2) + 1) * (NT // 8) * 384)
                        nc.scalar.dma_start(cos_sb[:, qq], cos3[:, qq])
                        nc.scalar.dma_start(sin_sb[:, qq], sin3[:, qq])
                    # cast to f16 on the (idle-in-stage-1) Pool engine, then
                    # batched PE transpose of the whole [128, C] chunk at the
                    # f16 transpose rate
                    xh_sb = s1.tile([128, C], F16, tag="xh", bufs=3)
                    nc.gpsimd.tensor_copy(xh_sb[:], x_sb[:].bitcast(F32))
                    xtp = s1ps.tile([128, C], F16, tag="xtp", bufs=2)
                    for c in range(NCC):
                        nc.tensor.transpose(xtp[:, c * 128:(c + 1) * 128],
                                            xh_sb[:, c * 128:(c + 1) * 128],
                                            identH[:])
                    xT_sb = s1.tile([128, C], F16, tag="xTs")
                    nc.vector.tensor_copy(xT_sb[:], xtp[:])
                    kv_ps = s1ps.tile([128, 2 * CG], F32, tag="kv")
                    for c in range(NCC):
                        nc.tensor.matmul(
                            kv_ps[:], xT_sb[:, c * 128:(c + 1) * 128],
                            wkv_sb[:, c * 2 * CG:(c + 1) * 2 * CG],
                            start=(c == 0), stop=(c == NCC - 1))
                    # K|V to SBUF in f16 (all downstream compute is f16)
                    kv_sb = s1.tile([128, 2 * CG], F16, tag="kvs")
                    nc.scalar.copy(kv_sb[:], kv_ps[:])

                    # RoPE on deinterleaved [re(32)|im(32)] blocks, K and Q
                    # halves in one 384-wide op each:
                    # K half -> kq[:, 0:CG], Q = rope(V) half -> kq[:, CG:2CG]
                    kq_sb = s1.tile([128, 2 * CG], F16, tag="kq")
                    cS = cos_sb[:, i * 2 * CG:(i + 1) * 2 * CG]
                    sS = sin_sb[:, i * 2 * CG:(i + 1) * 2 * CG]
                    a_sb = s1.tile([128, 2 * CG], F16, tag="ra")
                    b_sb = s1.tile([128, 2 * CG], F16, tag="rb")
                    nc.vector.tensor_mul(a_sb[:], kv_sb[:], cS)
                    nc.vector.tensor_mul(b_sb[:], kv_sb[:], sS)
                    a6 = a_sb[:].rearrange("p (h c) -> p h c", h=2 * HG)
                    b6 = b_sb[:].rearrange("p (h c) -> p h c", h=2 * HG)
                    o6 = kq_sb[:].rearrange("p (h c) -> p h c", h=2 * HG)
                    nc.vector.tensor_sub(o6[:, :, 0:32],
                                         a6[:, :, 0:32], b6[:, :, 32:64])
                    nc.vector.tensor_add(o6[:, :, 32:64],
                                         b6[:, :, 0:32], a6[:, :, 32:64])

                    # V (unroped, deinterleaved dims) into vaug [T, 65*3]
                    vdst = vaug[:, i * 195:(i + 1) * 195] \
                        .rearrange("p (h c) -> p h c", h=HG)[:, :, 0:64]
                    vsrc = kv_sb[:, CG:2 * CG].rearrange("p (h c) -> p h c", h=HG)
                    nc.gpsimd.tensor_copy(vdst, vsrc)

                    # transpose roped K and Q into [D, T] per-head layouts
                    tp = s1ps.tile([128, 4 * 128], F16, tag="tp")
                    nc.tensor.transpose(tp[:, 0:128], kq_sb[:, 0:128], identH[:])
                    nc.tensor.transpose(tp[:, 128:256], kq_sb[:, 192:320],
                                        identH[:])
                    nc.tensor.transpose(tp[0:64, 256:384], kq_sb[:, 128:192],
                                        identH[:])
                    nc.tensor.transpose(tp[0:64, 384:512], kq_sb[:, 320:384],
                                        identH[:])
                    ts_ = slice(i * 128, (i + 1) * 128)
                    nc.scalar.copy(kT01[:, ts_], tp[:, 0:128])
                    nc.scalar.copy(qT01[:, ts_], tp[:, 128:256])
                    nc.scalar.copy(kT2[:, ts_], tp[0:64, 256:384])
                    nc.scalar.copy(qT2[:, ts_], tp[0:64, 384:512])

              # projection weights load during stage 2
              nc.sync.dma_start(wp_sb[:].rearrange("p (n m) -> p n m", n=NCC),
                                wp.rearrange("(n p) m -> p n m", p=128))
              nc.sync.dma_start(bp_sb[:], bp)

              with tc.tile_pool(name="s23ps", bufs=2, space="PSUM") as s2ps:
                  # software pipeline over (split, head) pairs: pair j's score
                  # matmuls + exp issue BEFORE pair j-1's o-matmuls, so PE
                  # fills the exp latency with useful score work instead of
                  # stalling on the ACT engine per pack.
                  pairs = [(b, h) for b in range(len(SPLITS))
                           for h in range(HG)]

                  def _packs(bs, bw):
                      nblk = (bs + bw) // 128
                      packs, cur, w = [], [], 0
                      for t in range(nblk):
                          diag = t * 128 >= bs
                          col0 = t * 128 - bs if diag else 0
                          ncols = bw - col0
                          if w + ncols > 2 * TQ:
                              packs.append(cur)
                              cur, w = [], 0
                          cur.append((t, col0, ncols, w, diag))
                          w += ncols
                      if cur:
                          packs.append(cur)
                      return packs

                  def issue_scores(j):
                      b, h = pairs[j]
                      bs, bw = SPLITS[b]
                      kT = (kT01[0:64], kT01[64:128], kT2[0:64])[h]
                      qT = (qT01[0:64], qT01[64:128], qT2[0:64])[h]
                      out = []
                      for pk in _packs(bs, bw):
                          pw = sum(c[2] for c in pk)
                          s_ps = s2ps.tile([128, 2 * TQ], F32, tag="s", bufs=2)
                          wei = s2.tile([128, 2 * TQ], F16, tag="wei", bufs=8)
                          for t, col0, ncols, off, diag in pk:
                              nc.tensor.matmul(
                                  s_ps[:, off:off + ncols],
                                  kT[:, t * 128:(t + 1) * 128],
                                  qT[:, bs + col0:bs + bw],
                                  start=True, stop=True)
                          nc.scalar.activation(wei[:, 0:pw], s_ps[:, 0:pw],
                                               EXP, scale=SCALE)
                          out.append((pk, wei))
                      return out

                  def issue_o(j, scored):
                      b, h = pairs[j]
                      bs, bw = SPLITS[b]
                      oT = (oT01h[b][0:64], oT01h[b][64:128], oT2h[b][0:64])[h]
                      nblk = (bs + bw) // 128
                      o_ps = s2ps.tile([65, TQ], F32, tag="o", bufs=3)
                      for pk, wei in scored:
                          for t, col0, ncols, off, diag in pk:
                              if diag:
                                  nc.gpsimd.tensor_mul(wei[:, off:off + 128],
                                                       wei[:, off:off + 128],
                                                       triH[:])
                              va = t * 195 + h * 65
                              nc.tensor.matmul(
                                  o_ps[0:65, col0:bw], vaug[:, va:va + 65],
                                  wei[:, off:off + ncols],
                                  start=(t == 0), stop=(t == nblk - 1))
                      recip = s2.tile([1, TQ], F32, tag="recip", bufs=2)
                      nc.vector.reciprocal(recip[:, 0:bw], o_ps[64:65, 0:bw])
                      rb = s2.tile([64, TQ], F32, tag="rbd", bufs=2)
                      nc.gpsimd.partition_broadcast(rb[:, 0:bw],
                                                    recip[:, 0:bw])
                      nc.vector.tensor_mul(oT[:], o_ps[0:64, 0:bw],
                                           rb[:, 0:bw])
                      if h == HG - 1:
                          issue_stage3(b)

                  def issue_stage3(b):
                      # AllGather within the batch group, then column-sharded
                      # projection consuming the gathered f16 tiles as lhsT.
                      bs, bw = SPLITS[b]
                      nc.sync.dma_start(ag_in[b][0:64, :], oT01h[b][0:64])
                      nc.gpsimd.dma_start(ag_in[b][64:128, :], oT01h[b][64:128])
                      nc.scalar.dma_start(ag_in[b][128:CG, :], oT2h[b][:])
                      if sim_variant:
                          qs_ = (nc.sync, nc.scalar, nc.gpsimd, nc.sync)
                          for gg in range(G):
                              qs_[gg].dma_start(
                                  ag_out[b][gg * CG:(gg + 1) * CG, :],
                                  ag_in[b][:])
                      else:
                          nc.gpsimd.collective_compute(
                              "AllGather", mybir.AluOpType.bypass,
                              replica_groups=[[0, 1, 2, 3], [4, 5, 6, 7]],
                              ins=[ag_in[b][:].opt()], outs=[ag_out[b][:].opt()])
                      ntc = bw // 128
                      a_bf = s3.tile([128, NCC * TQ], F16, tag="abf", bufs=2)
                      nc.sync.dma_start(
                          a_bf[:, 0:NCC * bw].rearrange("p (n m) -> p n m", n=NCC),
                          ag_out[b][:].rearrange("(n p) m -> p n m", p=128))
                      o_sb = s3.tile([128, 4 * CG], F16, tag="osb", bufs=2)
                      for tc_ in range(ntc):
                          p_ps = s2ps.tile([128, CG], F32, tag="p", bufs=1)
                          for c in range(NCC):
                              nc.tensor.matmul(
                                  p_ps[:],
                                  a_bf[:, c * bw + tc_ * 128:
                                       c * bw + (tc_ + 1) * 128],
                                  wp_sb[:, c * CG:(c + 1) * CG],
                                  start=(c == 0), stop=False)
                          nc.tensor.matmul(p_ps[:], ones1[:], bp_sb[:],
                                           start=False, stop=True)
                          nc.vector.tensor_copy(
                              o_sb[:, tc_ * CG:(tc_ + 1) * CG], p_ps[:])
                      nc.sync.dma_start(
                          out_t[bs:bs + bw, :].rearrange("(n p) m -> p n m",
                                                         p=128),
                          o_sb[:, 0:ntc * CG].rearrange("p (n m) -> p n m",
                                                        n=ntc))

                  scored = {}
                  for j in range(len(pairs) + 1):
                      if j < len(pairs):
                          scored[j] = issue_scores(j)
                      if j >= 1:
                          issue_o(j - 1, scored.pop(j - 1))


def _build(sim_variant=False, reps=1):
    nc = bacc.Bacc("TRN2", target_bir_lowering=False, debug=False,
                   num_devices=1 if sim_variant else N_CORES,
                   enable_asserts=False)
    x = nc.dram_tensor("x", [T, C], F32, kind="ExternalInput").ap()
    wkv = nc.dram_tensor("wkv", [C, 2 * CG], F16, kind="ExternalInput").ap()
    wp = nc.dram_tensor("wp", [C, CG], F16, kind="ExternalInput").ap()
    bp = nc.dram_tensor("bp", [1, CG], F16, kind="ExternalInput").ap()
    cos3 = nc.dram_tensor("cos3", [128, NT * 384], F16, kind="ExternalInput").ap()
    sin3 = nc.dram_tensor("sin3", [128, NT * 384], F16, kind="ExternalInput").ap()
    out_t = nc.dram_tensor("out_t", [T, CG], F16, kind="ExternalOutput").ap()
    with tile.TileContext(nc) as tc:
        _body(nc, tc, x, wkv, wp, bp, cos3, sin3, out_t, sim_variant, reps)
    nc.compile()
    return nc


_NC = None


def _get_nc():
    global _NC
    if _NC is None:
        _NC = _build()
    return _NC


_EXEC = None


def _get_exec():
    global _EXEC
    if _EXEC is None:
        _EXEC = _make_exec(_get_nc())
    return _EXEC


def _make_exec(nc):
    """Reusable jitted SPMD executable (mirrors bass2jax.run_bass_via_pjrt's
    multi-core path)."""
    import jax
    from jax.experimental.shard_map import shard_map
    from jax.sharding import Mesh, PartitionSpec
    from concourse import bass2jax, mybir as _mybir

    bass2jax.install_neuronx_cc_hook()
    in_names, out_names, out_avals, zero_outs = [], [], [], []
    assert nc.dbg_addr is None
    pname = nc.partition_id_tensor.name if nc.partition_id_tensor else None
    for alloc in nc.m.functions[0].allocations:
        if not isinstance(alloc, _mybir.MemoryLocationSet):
            continue
        name = alloc.memorylocations[0].name
        if alloc.kind == "ExternalInput":
            if name != pname:
                in_names.append(name)
        elif alloc.kind == "ExternalOutput":
            out_names.append(name)
            shape = tuple(alloc.tensor_shape)
            dtype = _mybir.dt.np(alloc.dtype)
            out_avals.append(jax.core.ShapedArray(shape, dtype))
            zero_outs.append(np.zeros(shape, dtype))
    n_params = len(in_names)
    all_names = in_names + out_names
    if pname is not None:
        all_names = all_names + [pname]

    def _fn(*args):
        operands = list(args)
        if pname is not None:
            operands.append(bass2jax.partition_id_tensor())
        outs = bass2jax._bass_exec_p.bind(
            *operands,
            out_avals=tuple(out_avals),
            in_names=tuple(all_names),
            out_names=tuple(out_names),
            lowering_input_output_aliases=(),
            sim_require_finite=True,
            sim_require_nnan=True,
            nc=nc,
        )
        return tuple(outs)

    devices = jax.devices()[:N_CORES]
    mesh = Mesh(np.asarray(devices), ("core",))
    nin = n_params + len(out_names)
    donate = tuple(range(n_params, n_params + len(out_names)))
    sharded = jax.jit(
        shard_map(_fn, mesh=mesh,
                  in_specs=(PartitionSpec("core"),) * nin,
                  out_specs=(PartitionSpec("core"),) * len(out_names),
                  check_rep=False),
        donate_argnums=donate, keep_unused=True)

    def _zero_cat():
        return [np.zeros((N_CORES * z.shape[0], *z.shape[1:]), z.dtype)
                for z in zero_outs]

    return (sharded, in_names, out_names, out_avals, _zero_cat)


def _run_cached(in_maps):
    sharded, in_names, out_names, out_avals, zero_cat = _get_exec()
    concat_in = [np.concatenate([np.asarray(in_maps[c][n])
                                 for c in range(N_CORES)], axis=0)
                 for n in in_names]
    out_arrs = sharded(*concat_in, *zero_cat())
    return [
        {name: np.asarray(out_arrs[i]).reshape(N_CORES, *out_avals[i].shape)[c]
         for i, name in enumerate(out_names)}
        for c in range(N_CORES)
    ]


_PERM = np.concatenate([np.arange(0, 64, 2), np.arange(1, 64, 2)])  # [re|im]


def _prep_rope(r):
    # [T, 32] -> [128, NT*384] f16 deinterleaved and duplicated across the
    # K and Q halves: chunk i, half f, head h, cols i*384 + f*192 + h*64 +
    # {j, 32+j} both hold r[i*128+p, j]
    rr = r.reshape(NT, 128, 32).transpose(1, 0, 2)           # [128, NT, 32]
    rr = np.concatenate([rr, rr], axis=2)                    # [128, NT, 64]
    rr = np.broadcast_to(rr[:, :, None, :], (128, NT, 2 * HG, 64))
    return np.ascontiguousarray(rr.reshape(128, NT * 384), dtype=np.float16)


def _shard_inputs(x, rope_cos, rope_sin, W_att, W_proj, b_proj):
    x = np.ascontiguousarray(np.asarray(x, np.float32))
    W_att = np.asarray(W_att, np.float32)
    W_proj = np.asarray(W_proj, np.float32)
    b_proj = np.asarray(b_proj, np.float32)
    cos3 = _prep_rope(np.asarray(rope_cos, np.float32))
    sin3 = _prep_rope(np.asarray(rope_sin, np.float32))
    in_maps = []
    for r in range(N_CORES):
        b, g = divmod(r, G)
        c0 = g * CG
        # per-head deinterleave permutation of the group's 192 dims
        dperm = np.concatenate([h * 64 + _PERM for h in range(HG)])
        # full-C row permutation for W_proj: every group's dims are
        # deinterleaved in the gathered activation layout
        cperm = np.concatenate([g2 * CG + dperm for g2 in range(G)])
        wkv = np.ascontiguousarray(
            np.concatenate([W_att[:, c0:c0 + CG][:, dperm],
                            W_att[:, 2 * C + c0:2 * C + c0 + CG][:, dperm]],
                           axis=1)).astype(np.float16)
        in_maps.append({
            "x": x[b],
            "wkv": wkv,
            "wp": np.ascontiguousarray(
                W_proj[cperm, :][:, c0:c0 + CG]).astype(np.float16),
            "bp": np.ascontiguousarray(
                b_proj[c0:c0 + CG][None, :]).astype(np.float16),
            "cos3": cos3,
            "sin3": sin3,
        })
    return in_maps


def kernel(x, rope_cos, rope_sin, W_att, W_proj, b_proj, _run_kwargs=None):
    nc = _get_nc()
    in_maps = _shard_inputs(x, rope_cos, rope_sin, W_att, W_proj, b_proj)
    global _FIRST_CALL_DONE, _last_in_maps
    _last_in_maps = in_maps
    if not _FIRST_CALL_DONE:
        res = run_bass_kernel_spmd(nc, in_maps, core_ids=list(range(N_CORES)),
                                   **(_run_kwargs or {}))
        results = res.results
        kernel.last_results = res
        _FIRST_CALL_DONE = True
    else:
        results = _run_cached(in_maps)
    out = np.empty((B, T, C), np.float32)
    for r in range(N_CORES):
        b, g = divmod(r, G)
        out[b, :, g * CG:(g + 1) * CG] = results[r]["out_t"].astype(np.float32)
    return out


_FIRST_CALL_DONE = False


# revision 39
# speedup vs baseline: 1.0472x; 1.0387x over previous
"""Trainium2 Bass kernel for DecoderMultiHeadAttention (B=2, T=2048, C=768, H=12).

Sharding: 8 cores = 2 batches x 4 head-groups (3 heads each).
Per core: K,V projections for its head group (f32r), RoPE in f16 on
deinterleaved (real|imag) head layout, causal flash-style attention with
transposed score layout in f16, per-T-split AllGather within each batch
group of 4 cores overlapped with later attention splits, then a
column-sharded output projection consuming the gathered f16 tiles
directly as matmul lhsT (output in [T, 192] f16).

Host-side weight prep: W_att K/V columns and W_proj rows are permuted so
each head's dim order is [re(32)|im(32)] — keeps every RoPE vector op on
unit-stride 32-col blocks (DVE 4x f16 mode). Scores are invariant to the
shared K/Q permutation; W_proj rows are permuted to match V's order.

Note: the reference uses q = rope(v) (faithful source bug), so the
q-chunk of W_att (columns C..2C) is never used and is not computed.
"""

import sys

_REPO = "/opt/trn_rl_repo"
if _REPO not in sys.path:
    sys.path.insert(0, _REPO)

import numpy as np

import concourse.bass as bass
import concourse.mybir as mybir
import concourse.tile as tile
from concourse import bacc
from concourse.bass_utils import run_bass_kernel_spmd
from concourse.masks import make_identity

B, T, C, H = 2, 2048, 768, 12
D = C // H            # 64
N_CORES = 8
G = 4                 # head groups
HG = H // G           # 3 heads per group
CG = HG * D           # 192 output columns per group
NT = T // 128         # 16 t-chunks
NCC = C // 128        # 6 c-chunks
TQ = 512              # q block width
F32 = mybir.dt.float32
F32R = mybir.dt.float32r
F16 = mybir.dt.float16
EXP = mybir.ActivationFunctionType.Exp
SCALE = float(D) ** -0.5
SPLITS = [(0, 512), (512, 512), (1024, 512), (1536, 256), (1792, 256)]


def _body(nc, tc, x, wkv, wp, bp, cos3, sin3, out_t, sim_variant=False, reps=1):
    with tc.tile_pool(name="const", bufs=1) as cp:
        ident = cp.tile([128, 128], F32)
        make_identity(nc, ident[:])
        identR = cp.tile([128, 128], F32R)
        nc.scalar.copy(identR[:], ident[:])
        identH = cp.tile([128, 128], F16)
        nc.vector.tensor_copy(identH[:], ident[:])
        # tri[p, f] = 1.0 if f >= p else 0.0  (keep tq >= tk in diagonal blocks)
        tri = cp.tile([128, 128], F32)
        nc.gpsimd.memset(tri[:], 1.0)
        nc.gpsimd.affine_select(
            out=tri[:], in_=tri[:], compare_op=mybir.AluOpType.is_ge,
            fill=0.0, base=0, pattern=[[1, 128]], channel_multiplier=-1)
        triH = cp.tile([128, 128], F16)
        nc.vector.tensor_copy(triH[:], tri[:])

        # cdup/sdup: [128, NT*384] f16, deinterleaved + duplicated across the
        # K and Q halves: chunk i, half f, head h, cols i*384 + f*192 + h*64
        # + {j, 32+j} = table[i*128+p, j] — one 384-wide mul ropes K and Q
        cos_sb = cp.tile([128, NT * 384], F16)
        sin_sb = cp.tile([128, NT * 384], F16)
        # K,V weights in f16 (host-converted); stage-1 matmuls run f16
        wkv_sb = cp.tile([128, NCC * 2 * CG], F16)
        nc.scalar.dma_start(
            wkv_sb[:].rearrange("p (n m) -> p n m", n=NCC),
            wkv.rearrange("(n p) m -> p n m", p=128))
        # projection weights / bias in f16 (host-converted)
        wp_sb = cp.tile([128, NCC * CG], F16)
        bp_sb = cp.tile([1, CG], F16)
        ones1 = cp.tile([1, 128], F16)
        nc.gpsimd.memset(ones1[:], 1.0)

        # persistent per-head [D, T] f16 tensors: K in cols [0:T], Q in
        # cols [T:2T] of the same tile so each chunk's PSUM drain is one
        # batched copy; heads 0,1 stacked in partitions, head 2 separate
        kq01 = cp.tile([128, 2 * T], F16)
        kq2 = cp.tile([64, 2 * T], F16)
        # V in [T, D] layout with a ones column per head: per t-chunk i,
        # cols [i*195 + h*65 : +64] = V_h (deinterleaved dim order),
        # col +64 = 1.0
        vaug = cp.tile([128, NT * (HG * 65)], F16)
        ones48 = cp.tile([128, NT * HG], F16)
        nc.gpsimd.memset(ones48[:], 1.0)
        nc.vector.tensor_copy(
            vaug[:].rearrange("p (k c) -> p k c", c=65)[:, :, 64], ones48[:])
        # attention output, transposed [CG, bw] f16, one tile pair per split
        NSP = len(SPLITS)
        oT01h = [cp.tile([128, bw], F16, name=f"oT01h{k}")
                 for k, (bs_, bw) in enumerate(SPLITS)]
        oT2h = [cp.tile([64, bw], F16, name=f"oT2h{k}")
                for k, (bs_, bw) in enumerate(SPLITS)]

        dp = tc.alloc_tile_pool(name="dram", bufs=1, space="DRAM")
        ag_in = [dp.tile([CG, bw], F16, name=f"agin{k}")
                 for k, (bs_, bw) in enumerate(SPLITS)]
        ag_out = [dp.tile([G * CG, bw], F16, name=f"agout{k}")
                  for k, (bs_, bw) in enumerate(SPLITS)]

        for _rep in range(reps):
            # ---- Stage 1: KV projection + RoPE + transposes ----
            with tc.tile_pool(name="s1", bufs=3) as s1, \
                 tc.tile_pool(name="s2", bufs=2) as s2, \
                 tc.tile_pool(name="s3", bufs=1) as s3:
              with tc.tile_pool(name="s1ps", bufs=2, space="PSUM") as s1ps:
                for i in range(NT):
                    x_sb = s1.tile([128, C], F32R, tag="x", bufs=4)
                    nc.sync.dma_start(x_sb[:],
                                      x[i * 128:(i + 1) * 128, :].bitcast(F32R))
                    if i % 2 == 0 and i < 16:
                        # rope tables in eighths on the scalar HWDGE queue,
                        # spread so x-chunk loads are not starved early
                        qq = slice((i // 2) * (NT // 8) * 384,
                                   ((i // 2) + 1) * (NT // 8) * 384)
                        nc.scalar.dma_start(cos_sb[:, qq], cos3[:, qq])
                        nc.scalar.dma_start(sin_sb[:, qq], sin3[:, qq])
                    # cast to f16 on the (idle-in-stage-1) Pool engine, then
                    # batched PE transpose of the whole [128, C] chunk at the
                    # f16 transpose rate
                    xh_sb = s1.tile([128, C], F16, tag="xh", bufs=3)
                    nc.gpsimd.tensor_copy(xh_sb[:], x_sb[:].bitcast(F32))
                    xtp = s1ps.tile([128, C], F16, tag="xtp", bufs=2)
                    for c in range(NCC):
                        nc.tensor.transpose(xtp[:, c * 128:(c + 1) * 128],
                                            xh_sb[:, c * 128:(c + 1) * 128],
                                            identH[:])
                    xT_sb = s1.tile([128, C], F16, tag="xTs")
                    nc.vector.tensor_copy(xT_sb[:], xtp[:])
                    kv_ps = s1ps.tile([128, 2 * CG], F32, tag="kv")
                    for c in range(NCC):
                        nc.tensor.matmul(
                            kv_ps[:], xT_sb[:, c * 128:(c + 1) * 128],
                            wkv_sb[:, c * 2 * CG:(c + 1) * 2 * CG],
                            start=(c == 0), stop=(c == NCC - 1))
                    # K|V to SBUF in f16 (all downstream compute is f16)
                    kv_sb = s1.tile([128, 2 * CG], F16, tag="kvs")
                    nc.scalar.copy(kv_sb[:], kv_ps[:])

                    # RoPE on deinterleaved [re(32)|im(32)] blocks, K and Q
                    # halves in one 384-wide op each:
                    # K half -> kq[:, 0:CG], Q = rope(V) half -> kq[:, CG:2CG]
                    kq_sb = s1.tile([128, 2 * CG], F16, tag="kq")
                    cS = cos_sb[:, i * 2 * CG:(i + 1) * 2 * CG]
                    sS = sin_sb[:, i * 2 * CG:(i + 1) * 2 * CG]
                    a_sb = s1.tile([128, 2 * CG], F16, tag="ra")
                    b_sb = s1.tile([128, 2 * CG], F16, tag="rb")
                    nc.vector.tensor_mul(a_sb[:], kv_sb[:], cS)
                    nc.vector.tensor_mul(b_sb[:], kv_sb[:], sS)
                    a6 = a_sb[:].rearrange("p (h c) -> p h c", h=2 * HG)
                    b6 = b_sb[:].rearrange("p (h c) -> p h c", h=2 * HG)
                    o6 = kq_sb[:].rearrange("p (h c) -> p h c", h=2 * HG)
                    nc.vector.tensor_sub(o6[:, :, 0:32],
                                         a6[:, :, 0:32], b6[:, :, 32:64])
                    nc.vector.tensor_add(o6[:, :, 32:64],
                                         b6[:, :, 0:32], a6[:, :, 32:64])

                    # V (unroped, deinterleaved dims) into vaug [T, 65*3]
                    vdst = vaug[:, i * 195:(i + 1) * 195] \
                        .rearrange("p (h c) -> p h c", h=HG)[:, :, 0:64]
                    vsrc = kv_sb[:, CG:2 * CG].rearrange("p (h c) -> p h c", h=HG)
                    nc.gpsimd.tensor_copy(vdst, vsrc)

                    # transpose roped K and Q into [D, T] per-head layouts
                    tp = s1ps.tile([128, 4 * 128], F16, tag="tp")
                    nc.tensor.transpose(tp[:, 0:128], kq_sb[:, 0:128], identH[:])
                    nc.tensor.transpose(tp[:, 128:256], kq_sb[:, 192:320],
                                        identH[:])
                    nc.tensor.transpose(tp[0:64, 256:384], kq_sb[:, 128:192],
                                        identH[:])
                    nc.tensor.transpose(tp[0:64, 384:512], kq_sb[:, 320:384],
                                        identH[:])
                    d01 = kq01[:].rearrange("p (s t) -> p s t", s=2)[
                        :, :, i * 128:(i + 1) * 128]
                    nc.scalar.copy(
                        d01, tp[:, 0:256].rearrange("p (s t) -> p s t", s=2))
                    d2 = kq2[:].rearrange("p (s t) -> p s t", s=2)[
                        :, :, i * 128:(i + 1) * 128]
                    nc.scalar.copy(
                        d2, tp[0:64, 256:512].rearrange("p (s t) -> p s t",
                                                        s=2))

              # projection weights load during stage 2
              nc.sync.dma_start(wp_sb[:].rearrange("p (n m) -> p n m", n=NCC),
                                wp.rearrange("(n p) m -> p n m", p=128))
              nc.sync.dma_start(bp_sb[:], bp)

              with tc.tile_pool(name="s23ps", bufs=2, space="PSUM") as s2ps:
                  # software pipeline over (split, head) pairs: pair j's score
                  # matmuls + exp issue BEFORE pair j-1's o-matmuls, so PE
                  # fills the exp latency with useful score work instead of
                  # stalling on the ACT engine per pack.
                  pairs = [(b, h) for b in range(len(SPLITS))
                           for h in range(HG)]

                  def _packs(bs, bw):
                      nblk = (bs + bw) // 128
                      packs, cur, w = [], [], 0
                      for t in range(nblk):
                          diag = t * 128 >= bs
                          col0 = t * 128 - bs if diag else 0
                          ncols = bw - col0
                          if w + ncols > 2 * TQ:
                              packs.append(cur)
                              cur, w = [], 0
                          cur.append((t, col0, ncols, w, diag))
                          w += ncols
                      if cur:
                          packs.append(cur)
                      return packs

                  def issue_scores(j):
                      b, h = pairs[j]
                      bs, bw = SPLITS[b]
                      kq = (kq01[0:64], kq01[64:128], kq2[0:64])[h]
                      out = []
                      for pk in _packs(bs, bw):
                          pw = sum(c[2] for c in pk)
                          s_ps = s2ps.tile([128, 2 * TQ], F32, tag="s", bufs=2)
                          wei = s2.tile([128, 2 * TQ], F16, tag="wei", bufs=8)
                          for t, col0, ncols, off, diag in pk:
                              nc.tensor.matmul(
                                  s_ps[:, off:off + ncols],
                                  kq[:, t * 128:(t + 1) * 128],
                                  kq[:, T + bs + col0:T + bs + bw],
                                  start=True, stop=True)
                          nc.scalar.activation(wei[:, 0:pw], s_ps[:, 0:pw],
                                               EXP, scale=SCALE)
                          out.append((pk, wei))
                      return out

                  def issue_o(j, scored):
                      b, h = pairs[j]
                      bs, bw = SPLITS[b]
                      oT = (oT01h[b][0:64], oT01h[b][64:128], oT2h[b][0:64])[h]
                      nblk = (bs + bw) // 128
                      o_ps = s2ps.tile([65, TQ], F32, tag="o", bufs=3)
                      for pk, wei in scored:
                          for t, col0, ncols, off, diag in pk:
                              if diag:
                                  nc.gpsimd.tensor_mul(wei[:, off:off + 128],
                                                       wei[:, off:off + 128],
                                                       triH[:])
                              va = t * 195 + h * 65
                              nc.tensor.matmul(
                                  o_ps[0:65, col0:bw], vaug[:, va:va + 65],
                                  wei[:, off:off + ncols],
                                  start=(t == 0), stop=(t == nblk - 1))
                      recip = s2.tile([1, TQ], F32, tag="recip", bufs=2)
                      nc.vector.reciprocal(recip[:, 0:bw], o_ps[64:65, 0:bw])
                      rb = s2.tile([64, TQ], F32, tag="rbd", bufs=2)
                      nc.gpsimd.partition_broadcast(rb[:, 0:bw],
                                                    recip[:, 0:bw])
                      nc.vector.tensor_mul(oT[:], o_ps[0:64, 0:bw],
                                           rb[:, 0:bw])
                      if h == HG - 1:
                          issue_stage3(b)

                  def issue_stage3(b):
                      # AllGather within the batch group, then column-sharded
                      # projection consuming the gathered f16 tiles as lhsT.
                      bs, bw = SPLITS[b]
                      nc.sync.dma_start(ag_in[b][0:64, :], oT01h[b][0:64])
                      nc.gpsimd.dma_start(ag_in[b][64:128, :], oT01h[b][64:128])
                      nc.scalar.dma_start(ag_in[b][128:CG, :], oT2h[b][:])
                      if sim_variant:
                          qs_ = (nc.sync, nc.scalar, nc.gpsimd, nc.sync)
                          for gg in range(G):
                              qs_[gg].dma_start(
                                  ag_out[b][gg * CG:(gg + 1) * CG, :],
                                  ag_in[b][:])
                      else:
                          nc.gpsimd.collective_compute(
                              "AllGather", mybir.AluOpType.bypass,
                              replica_groups=[[0, 1, 2, 3], [4, 5, 6, 7]],
                              ins=[ag_in[b][:].opt()], outs=[ag_out[b][:].opt()])
                      ntc = bw // 128
                      a_bf = s3.tile([128, NCC * TQ], F16, tag="abf", bufs=2)
                      nc.sync.dma_start(
                          a_bf[:, 0:NCC * bw].rearrange("p (n m) -> p n m", n=NCC),
                          ag_out[b][:].rearrange("(n p) m -> p n m", p=128))
                      o_sb = s3.tile([128, 4 * CG], F16, tag="osb", bufs=2)
                      for tc_ in range(ntc):
                          p_ps = s2ps.tile([128, CG], F32, tag="p", bufs=1)
                          for c in range(NCC):
                              nc.tensor.matmul(
                                  p_ps[:],
                                  a_bf[:, c * bw + tc_ * 128:
                                       c * bw + (tc_ + 1) * 128],
                                  wp_sb[:, c * CG:(c + 1) * CG],
                                  start=(c == 0), stop=False)
                          nc.tensor.matmul(p_ps[:], ones1[:], bp_sb[:],
                                           start=False, stop=True)
                          nc.vector.tensor_copy(
                              o_sb[:, tc_ * CG:(tc_ + 1) * CG], p_ps[:])
                      nc.sync.dma_start(
                          out_t[bs:bs + bw, :].rearrange("(n p) m -> p n m",
                                                         p=128),
                          o_sb[:, 0:ntc * CG].rearrange("p (n m) -> p n m",
                                                        n=ntc))

                  scored = {}
                  for j in range(len(pairs) + 1):
                      if j < len(pairs):
                          scored[j] = issue_scores(j)
                      if j >= 1:
                          issue_o(j - 1, scored.pop(j - 1))


def _build(sim_variant=False, reps=1):
    nc = bacc.Bacc("TRN2", target_bir_lowering=False, debug=False,
                   num_devices=1 if sim_variant else N_CORES,
                   enable_asserts=False)
    x = nc.dram_tensor("x", [T, C], F32, kind="ExternalInput").ap()
    wkv = nc.dram_tensor("wkv", [C, 2 * CG], F16, kind="ExternalInput").ap()
    wp = nc.dram_tensor("wp", [C, CG], F16, kind="ExternalInput").ap()
    bp = nc.dram_tensor("bp", [1, CG], F16, kind="ExternalInput").ap()
    cos3 = nc.dram_tensor("cos3", [128, NT * 384], F16, kind="ExternalInput").ap()
    sin3 = nc.dram_tensor("sin3", [128, NT * 384], F16, kind="ExternalInput").ap()
    out_t = nc.dram_tensor("out_t", [T, CG], F16, kind="ExternalOutput").ap()
    with tile.TileContext(nc) as tc:
        _body(nc, tc, x, wkv, wp, bp, cos3, sin3, out_t, sim_variant, reps)
    nc.compile()
    return nc


_NC = None


def _get_nc():
    global _NC
    if _NC is None:
        _NC = _build()
    return _NC


_EXEC = None


def _get_exec():
    global _EXEC
    if _EXEC is None:
        _EXEC = _make_exec(_get_nc())
    return _EXEC


def _make_exec(nc):
    """Reusable jitted SPMD executable (mirrors bass2jax.run_bass_via_pjrt's
    multi-core path)."""
    import jax
    from jax.experimental.shard_map import shard_map
    from jax.sharding import Mesh, PartitionSpec
    from concourse import bass2jax, mybir as _mybir

    bass2jax.install_neuronx_cc_hook()
    in_names, out_names, out_avals, zero_outs = [], [], [], []
    assert nc.dbg_addr is None
    pname = nc.partition_id_tensor.name if nc.partition_id_tensor else None
    for alloc in nc.m.functions[0].allocations:
        if not isinstance(alloc, _mybir.MemoryLocationSet):
            continue
        name = alloc.memorylocations[0].name
        if alloc.kind == "ExternalInput":
            if name != pname:
                in_names.append(name)
        elif alloc.kind == "ExternalOutput":
            out_names.append(name)
            shape = tuple(alloc.tensor_shape)
            dtype = _mybir.dt.np(alloc.dtype)
            out_avals.append(jax.core.ShapedArray(shape, dtype))
            zero_outs.append(np.zeros(shape, dtype))
    n_params = len(in_names)
    all_names = in_names + out_names
    if pname is not None:
        all_names = all_names + [pname]

    def _fn(*args):
        operands = list(args)
        if pname is not None:
            operands.append(bass2jax.partition_id_tensor())
        outs = bass2jax._bass_exec_p.bind(
            *operands,
            out_avals=tuple(out_avals),
            in_names=tuple(all_names),
            out_names=tuple(out_names),
            lowering_input_output_aliases=(),
            sim_require_finite=True,
            sim_require_nnan=True,
            nc=nc,
        )
        return tuple(outs)

    devices = jax.devices()[:N_CORES]
    mesh = Mesh(np.asarray(devices), ("core",))
    nin = n_params + len(out_names)
    donate = tuple(range(n_params, n_params + len(out_names)))
    sharded = jax.jit(
        shard_map(_fn, mesh=mesh,
                  in_specs=(PartitionSpec("core"),) * nin,
                  out_specs=(PartitionSpec("core"),) * len(out_names),
                  check_rep=False),
        donate_argnums=donate, keep_unused=True)

    def _zero_cat():
        return [np.zeros((N_CORES * z.shape[0], *z.shape[1:]), z.dtype)
                for z in zero_outs]

    return (sharded, in_names, out_names, out_avals, _zero_cat)


def _run_cached(in_maps):
    sharded, in_names, out_names, out_avals, zero_cat = _get_exec()
    concat_in = [np.concatenate([np.asarray(in_maps[c][n])
                                 for c in range(N_CORES)], axis=0)
                 for n in in_names]
    out_arrs = sharded(*concat_in, *zero_cat())
    return [
        {name: np.asarray(out_arrs[i]).reshape(N_CORES, *out_avals[i].shape)[c]
         for i, name in enumerate(out_names)}
        for c in range(N_CORES)
    ]


_PERM = np.concatenate([np.arange(0, 64, 2), np.arange(1, 64, 2)])  # [re|im]


def _prep_rope(r):
    # [T, 32] -> [128, NT*384] f16 deinterleaved and duplicated across the
    # K and Q halves: chunk i, half f, head h, cols i*384 + f*192 + h*64 +
    # {j, 32+j} both hold r[i*128+p, j]
    rr = r.reshape(NT, 128, 32).transpose(1, 0, 2)           # [128, NT, 32]
    rr = np.concatenate([rr, rr], axis=2)                    # [128, NT, 64]
    rr = np.broadcast_to(rr[:, :, None, :], (128, NT, 2 * HG, 64))
    return np.ascontiguousarray(rr.reshape(128, NT * 384), dtype=np.float16)


def _shard_inputs(x, rope_cos, rope_sin, W_att, W_proj, b_proj):
    x = np.ascontiguousarray(np.asarray(x, np.float32))
    W_att = np.asarray(W_att, np.float32)
    W_proj = np.asarray(W_proj, np.float32)
    b_proj = np.asarray(b_proj, np.float32)
    cos3 = _prep_rope(np.asarray(rope_cos, np.float32))
    sin3 = _prep_rope(np.asarray(rope_sin, np.float32))
    in_maps = []
    for r in range(N_CORES):
        b, g = divmod(r, G)
        c0 = g * CG
        # per-head deinterleave permutation of the group's 192 dims
        dperm = np.concatenate([h * 64 + _PERM for h in range(HG)])
        # full-C row permutation for W_proj: every group's dims are
        # deinterleaved in the gathered activation layout
        cperm = np.concatenate([g2 * CG + dperm for g2 in range(G)])
        wkv = np.ascontiguousarray(
            np.concatenate([W_att[:, c0:c0 + CG][:, dperm],
                            W_att[:, 2 * C + c0:2 * C + c0 + CG][:, dperm]],
                           axis=1)).astype(np.float16)
        in_maps.append({
            "x": x[b],
            "wkv": wkv,
            "wp": np.ascontiguousarray(
                W_proj[cperm, :][:, c0:c0 + CG]).astype(np.float16),
            "bp": np.ascontiguousarray(
                b_proj[c0:c0 + CG][None, :]).astype(np.float16),
            "cos3": cos3,
            "sin3": sin3,
        })
    return in_maps


def kernel(x, rope_cos, rope_sin, W_att, W_proj, b_proj, _run_kwargs=None):
    nc = _get_nc()
    in_maps = _shard_inputs(x, rope_cos, rope_sin, W_att, W_proj, b_proj)
    global _FIRST_CALL_DONE, _last_in_maps
    _last_in_maps = in_maps
    if not _FIRST_CALL_DONE:
        res = run_bass_kernel_spmd(nc, in_maps, core_ids=list(range(N_CORES)),
                                   **(_run_kwargs or {}))
        results = res.results
        kernel.last_results = res
        _FIRST_CALL_DONE = True
    else:
        results = _run_cached(in_maps)
    out = np.empty((B, T, C), np.float32)
    for r in range(N_CORES):
        b, g = divmod(r, G)
        out[b, :, g * CG:(g + 1) * CG] = results[r]["out_t"].astype(np.float32)
    return out


_FIRST_CALL_DONE = False
